# revision 17
# baseline (speedup 1.0000x reference)
"""Trainium2 Bass kernel for nn_MultiHeadAttention (B=4, T=2048, D=1024,
H=16, d_k=64) on 8 NeuronCores.

Sharding: tensor-parallel over heads — core c computes heads {2c, 2c+1} for
ALL batches (W_q/W_k/W_v column-sharded, W_o row-sharded). The final
all-reduce of the output projection is replaced by a host-side sum of the 8
partial outputs (each written transposed, [D, T]).

v2 redesign (from perfetto/NTFF evidence of the v1 kernel):
  - K/V are projected (and their x DMA'd) only for the ceil(vl/128) Tk
    tiles that attention actually reads — 24 of 64 tiles for this seed.
  - Weight-stationary loop order for the Q/K projections and the output
    projection (wo stationary, partial output written as [D, T]), so
    LDWEIGHTS is amortized over 2-4 N=512 streams instead of one.
  - exp() over merged [128, 2x512] PSUM pairs: interior Tk tiles carry no
    mask, so one ACT instruction covers two tiles (the 352-cycle ACT
    overhead was 40% of exp cost); only the last, partially-masked tile
    keeps its per-partition bias operand.
  - softmax denominator row is DMA'd straight out of PSUM (ones-column
    folded into the P@V matmul as in v1), normalization is batched.
  - proj/norm/out-proj instruction emission is pumped in small units
    between attention j-iterations: the per-engine queues are in-order,
    so this is what lets the PE fill the gaps while ACT paces attention.
"""
import os
import sys
from collections import deque

for _p in ("/opt/trn_rl_repo", "/root/.axon_site/_ro/trn_rl_repo"):
    if os.path.isdir(_p) and _p not in sys.path:
        sys.path.append(_p)

import numpy as np
import ml_dtypes

import concourse.bass as bass
import concourse.mybir as mybir
import concourse.tile as tile
from concourse.bass import ts
from concourse.bass_utils import run_bass_kernel_spmd

D = 1024
T = 2048
H = 16
DK = 64
P = 128
KC = D // P          # 8 contraction chunks for the projections
TC = T // P          # 16 token tiles of 128
NT = T // 512        # 4 Tq chunks of 512
NCORES = 8
MASK_NEG = -30000.0

F32 = mybir.dt.float32
F32R = mybir.dt.float32r
BF16 = mybir.dt.bfloat16
AF = mybir.ActivationFunctionType
BF16_NP = ml_dtypes.bfloat16


def _split_multi_waits(nc):
    """trn2 instructions encode at most one sync wait; split the rest into
    standalone single-wait event-semaphore ops."""
    n_split = 0
    for f in nc.m.functions:
        for blk in f.blocks:
            insts = blk.instructions
            out = []
            changed = False
            for inst in insts:
                si = inst.sync_info
                if si is not None and len(si.on_wait) > 1:
                    waits = list(si.on_wait)
                    for k, wt in enumerate(waits[:-1]):
                        ev = mybir.InstEventSemaphore(
                            name=f"{inst.name}_wsplit{k}",
                            engine=inst.engine,
                            ins=[],
                            outs=[],
                            bass_nofuse=True,
                            sync_info=mybir.SyncInfo(on_wait=[wt], on_update=[]),
                        )
                        out.append(ev)
                        n_split += 1
                    inst.sync_info = mybir.SyncInfo(
                        on_wait=[waits[-1]], on_update=si.on_update
                    )
                    changed = True
                out.append(inst)
            if changed:
                blk.instructions = out
    return n_split


def build_nc(NB, J_list, dt_x):
    """Build the SPMD program.

    NB     : number of batch slots handled per core
    J_list : per batch slot, number of 128-row Tk tiles of attention
    dt_x   : dtype of x/weights/intermediates
    """
    nc = bass.Bass()

    # partition-major tile layout: per partition, each 128-token tile is a
    # contiguous [KC, 128] run (2 KB bf16)
    xq_d = [nc.declare_dram_parameter(f"xq{s}", [P, TC, KC, P], dt_x,
                                      isOutput=False) for s in range(NB)]
    xk_d = [nc.declare_dram_parameter(f"xk{s}", [P, J_list[s], KC, P], dt_x,
                                      isOutput=False) for s in range(NB)]
    xv_d = [nc.declare_dram_parameter(f"xv{s}", [P, J_list[s], KC, P], dt_x,
                                      isOutput=False) for s in range(NB)]
    wq_d = nc.declare_dram_parameter("wq", [P, KC, P], dt_x, isOutput=False)
    wk_d = nc.declare_dram_parameter("wk", [P, KC, P], dt_x, isOutput=False)
    wv_d = nc.declare_dram_parameter("wv", [P, KC, P], dt_x, isOutput=False)
    wo_d = nc.declare_dram_parameter("wo", [P, KC, P], dt_x, isOutput=False)
    bq_d = nc.declare_dram_parameter("bq", [P, 1], F32, isOutput=False)
    bk_d = nc.declare_dram_parameter("bk", [P, 1], F32, isOutput=False)
    bv_d = nc.declare_dram_parameter("bv", [1, P], dt_x, isOutput=False)
    mb_d = [nc.declare_dram_parameter(f"mb{s}", [1, P], BF16, isOutput=False)
            for s in range(NB)]
    onesb_d = nc.declare_dram_parameter("onesb", [1, DK], BF16,
                                        isOutput=False)
    o_d = [nc.declare_dram_parameter(f"o{s}", [D, T], BF16, isOutput=True)
           for s in range(NB)]

    with tile.TileContext(nc) as tc:
        with (
            tc.tile_pool(name="pers", bufs=1) as pers,
            tc.tile_pool(name="stream", bufs=3) as stream,
            tc.tile_pool(name="attn", bufs=3) as attn_pool,
            tc.tile_pool(name="small", bufs=3) as small,
            tc.tile_pool(name="outp", bufs=4) as outp,
            tc.tile_pool(name="ps_qk", bufs=2, space="PSUM") as ps_qk,
            tc.tile_pool(name="ps_pv", bufs=2, space="PSUM") as ps_pv,
            tc.tile_pool(name="ps_bg", bufs=2, space="PSUM") as ps_bg,
        ):
            # ---- persistent tensors -------------------------------------
            wq = pers.tile([P, KC, P], dt_x, name="wq")
            wk = pers.tile([P, KC, P], dt_x, name="wk")
            wv = pers.tile([P, KC, P], dt_x, name="wv")
            wo = pers.tile([P, KC, P], dt_x, name="wo")
            bq = pers.tile([P, 1], F32, name="bq")
            bk = pers.tile([P, 1], F32, name="bk")
            bv = pers.tile([1, P], dt_x, name="bv")
            nc.sync.dma_start(wq[:], wq_d[:])
            nc.sync.dma_start(wk[:], wk_d[:])
            nc.sync.dma_start(wv[:], wv_d[:])
            nc.sync.dma_start(wo[:], wo_d[:])
            nc.sync.dma_start(bq[:], bq_d[:])
            nc.sync.dma_start(bk[:], bk_d[:])
            nc.sync.dma_start(bv[:], bv_d[:])
            mb = []
            for s in range(NB):
                t = pers.tile([1, P], BF16, name=f"mb{s}")
                nc.sync.dma_start(t[:], mb_d[s][:])
                mb.append(t)
            ones_r = pers.tile([1, 512], BF16, name="ones_r")
            nc.vector.memset(ones_r[:], 1.0)

            ones_t = pers.tile([1, P], dt_x, name="ones_t")   # V-bias fold lhsT
            nc.vector.memset(ones_t[:], 1.0)
            ones_b = pers.tile([1, DK], BF16, name="ones_b")  # 1/den bcast lhsT
            nc.sync.dma_start(ones_b[:], onesb_d[:])

            QT = [pers.tile([P, T], dt_x, name=f"QT{s}") for s in range(NB)]
            KT = [pers.tile([P, J_list[s] * P], dt_x, name=f"KT{s}")
                  for s in range(NB)]
            # V with a ones column folded in at free index 64 of each head
            V = [pers.tile([P, J_list[s], 2, DK + 1], dt_x, name=f"V{s}")
                 for s in range(NB)]
            for s in range(NB):
                nc.vector.memset(V[s][:, :, :, DK], 1.0)

            AO = [pers.tile([P, T], dt_x, name=f"AO{s}") for s in range(NB)]
            NR = 2 * NT  # unnormalized-output rows per slot (tq, head)
            uo = [pers.tile([DK + 1, NR, 512], BF16, name=f"uo{s}")
                  for s in range(NB)]
            dens = [pers.tile([NR, 512], BF16, name=f"dens{s}")
                    for s in range(NB)]
            recs = [pers.tile([NR, 512], BF16, name=f"rec{s}")
                    for s in range(NB)]

            # ---- emission-unit generators -------------------------------
            def proj_gen(s):
                J = J_list[s]
                # K projection: weight-stationary per kc over <=4-tile chunks
                ngr = -(-J // 4)
                for g in range(ngr):
                    t0 = 4 * g
                    nt_ = min(4, J - t0)
                    xkw = stream.tile([P, 4, KC, P], dt_x, tag="xk_w")
                    nc.sync.dma_start(xkw[:, 0:nt_], xk_d[s][:, t0:t0 + nt_])
                    ps_k = ps_bg.tile([P, 512], F32, tag="bg")
                    for kc in range(KC):
                        nc.tensor.matmul(ps_k[:, 0:nt_ * P], wk[:, kc, :],
                                         xkw[:, 0:nt_, kc, :],
                                         start=(kc == 0), stop=(kc == KC - 1))
                        if kc == 3:
                            yield
                    nc.vector.tensor_scalar_add(
                        KT[s][:, t0 * P:(t0 + nt_) * P],
                        ps_k[:, 0:nt_ * P], bk[:, 0:1])
                    yield
                # V projection: x-tile stationary, wv moving (N=128)
                for g in range(ngr):
                    t0 = 4 * g
                    nt_ = min(4, J - t0)
                    xvw = stream.tile([P, 4, KC, P], dt_x, tag="xv_w")
                    nc.sync.dma_start(xvw[:, 0:nt_], xv_d[s][:, t0:t0 + nt_])
                    ps_v = ps_bg.tile([P, 512], F32, tag="bg")
                    for i in range(nt_):
                        reg = ps_v[:, ts(i, P)]
                        for kc in range(KC):
                            nc.tensor.matmul(reg, xvw[:, i, kc, :],
                                             wv[:, kc, :],
                                             start=(kc == 0), stop=False)
                        nc.tensor.matmul(reg, ones_t[0:1, :], bv[0:1, :],
                                         start=False, stop=True)
                        yield
                    for i in range(nt_):
                        nc.vector.tensor_copy(
                            V[s][:, t0 + i, :, 0:DK],
                            ps_v[:, ts(i, P)].rearrange("p (h d) -> p h d",
                                                        d=DK))
                    yield
                # Q projection: weight-stationary per kc over 4-tile chunks
                for g in range(NT):
                    xqw = stream.tile([P, 4, KC, P], dt_x, tag="xq_w")
                    nc.sync.dma_start(xqw[:], xq_d[s][:, 4 * g:4 * g + 4])
                    ps_q = ps_bg.tile([P, 512], F32, tag="bg")
                    for kc in range(KC):
                        nc.tensor.matmul(ps_q[:], wq[:, kc, :],
                                         xqw[:, :, kc, :],
                                         start=(kc == 0), stop=(kc == KC - 1))
                        if kc == 3:
                            yield
                    nc.vector.tensor_scalar_add(QT[s][:, ts(g, 512)],
                                                ps_q[:], bq[:, 0:1])
                    yield

            def attn_emit(s, pump):
                J = J_list[s]
                # j-tile pair schedule (the last tile's mask is premixed
                # into its psum, so it merges like any interior tile)
                items = [tuple(range(j, min(j + 2, J)))
                         for j in range(0, J, 2)]
                for tq in range(NT):
                    ps_os = [ps_pv.tile([P, 512], F32, tag="pv",
                                        name=f"pv{h}")
                             for h in range(2)]
                    # unit = (item, head): emit QK+exp for unit u, then the
                    # PV of unit u-1, so the PE never heads-of-line-waits on
                    # an exp (per-engine queues are in-order)
                    pv_pending = deque()
                    for it in items:
                        for h in range(2):
                            pss = ps_qk.tile([P, 2, 512], F32, tag="qk")
                            for k, j_ in enumerate(it):
                                masked = j_ == J - 1
                                nc.tensor.matmul(
                                    pss[:, k, :],
                                    KT[s][ts(h, DK), ts(j_, P)],
                                    QT[s][ts(h, DK), ts(tq, 512)],
                                    start=True, stop=not masked,
                                    tile_position=(h * DK, 0))
                                if masked:
                                    nc.tensor.matmul(
                                        pss[:, k, :], mb[s][0:1, :],
                                        ones_r[0:1, :],
                                        start=False, stop=True)
                            at = attn_pool.tile([P, 2, 512], dt_x, tag="at")
                            if len(it) == 2:
                                nc.scalar.activation(at[:, :, :], pss[:],
                                                     AF.Exp, scale=0.125)
                            else:
                                nc.scalar.activation(at[:, 0, :],
                                                     pss[:, 0, :], AF.Exp,
                                                     scale=0.125)
                            for k, j_ in enumerate(it):
                                pv_pending.append((j_, h, at, k))
                            while len(pv_pending) > 2:
                                _emit_pv(s, tq, ps_os, pv_pending.popleft())
                            pump()
                    while pv_pending:
                        _emit_pv(s, tq, ps_os, pv_pending.popleft())
                    for h in range(2):
                        r = tq * 2 + h
                        # den row rides along at partition DK (bf16 is
                        # plenty for the softmax denominator)
                        nc.vector.tensor_copy(uo[s][:, r, :],
                                              ps_os[h][0:DK + 1, :])
                        nc.sync.dma_start(dens[s][r:r + 1, :],
                                          uo[s][DK:DK + 1, r, :])
                    pump()

            def _emit_pv(s, tq, ps_os, unit):
                J = J_list[s]
                j_, h, at, k = unit
                nc.tensor.matmul(ps_os[h][0:DK + 1, :],
                                 V[s][:, j_, h, :], at[:, k, :],
                                 start=(j_ == 0), stop=(j_ == J - 1))

            def norm_gen(s):
                # batched normalization (bf16 denominators are plenty)
                with nc.allow_low_precision(reason="bf16 1/den is plenty"):
                    nc.vector.reciprocal(recs[s][:], dens[s][:])
                yield
                for r in range(NR):
                    # stage rec row at partition 0 for the K=1 bcast matmul
                    rst = small.tile([1, 512], BF16, tag="rst")
                    nc.sync.dma_start(rst[:], recs[s][r:r + 1, :])
                    ps_b = ps_bg.tile([P, 512], F32, tag="bg")
                    nc.tensor.matmul(ps_b[0:DK, :], ones_b[0:1, :],
                                     rst[0:1, :], start=True, stop=True)
                    h = r % 2
                    tq = r // 2
                    nc.vector.tensor_mul(
                        out=AO[s][ts(h, DK), ts(tq, 512)],
                        in0=ps_b[0:DK, :], in1=uo[s][0:DK, r, :])
                    yield

            def outproj_gen(s, use_act):
                # output projection, pumped into later attention windows;
                # copies go to ACT only where the exp stream is sparse
                unit = 0
                for dt_i in range(KC):
                    for n in range(NT):
                        ps_op = ps_bg.tile([P, 512], F32, tag="bg")
                        nc.tensor.matmul(ps_op[:], wo[:, dt_i, :],
                                         AO[s][:, ts(n, 512)],
                                         start=True, stop=True)
                        ot = outp.tile([P, 512], BF16, tag="ot")
                        if use_act and unit % 2 == 0:
                            nc.scalar.activation(ot[:], ps_op[:],
                                                 AF.Identity)
                        else:
                            nc.vector.tensor_copy(ot[:], ps_op[:])
                        nc.sync.dma_start(o_d[s][ts(dt_i, P), ts(n, 512)],
                                          ot[:])
                        unit += 1
                        yield

            # ---- software-pipelined emission ----------------------------
            pending = deque()

            def pump(n=2):
                k = 0
                while pending and k < n:
                    try:
                        next(pending[0][1])
                        k += 1
                    except StopIteration:
                        pending.popleft()

            def drain_proj():
                while pending and pending[0][0] == "proj":
                    try:
                        next(pending[0][1])
                    except StopIteration:
                        pending.popleft()

            for u in proj_gen(0):
                pass
            for s in range(NB):
                if s + 1 < NB:
                    pending.append(("proj", proj_gen(s + 1)))
                if s >= 1:
                    pending.append(("norm", norm_gen(s - 1)))
                    # ACT helps with copies only in exp-sparse windows
                    pending.append(("op", outproj_gen(s - 1,
                                                     J_list[s] < 8)))
                attn_emit(s, pump)
                drain_proj()
            pending.append(("norm", norm_gen(NB - 1)))
            pending.append(("op", outproj_gen(NB - 1, True)))
            while pending:
                pump(1000)

    _split_multi_waits(nc)
    return nc


_CACHE = {}


def _get_nc(NB, J_list, dt_x):
    key = (NB, tuple(J_list), str(dt_x))
    if key not in _CACHE:
        _CACHE[key] = build_nc(NB, J_list, dt_x)
    return _CACHE[key]


def _xt(x, dt_np, ntiles=TC):
    """[T, D] -> [P, ntiles, KC, 128] partition-major tile layout."""
    xt = x.T.reshape(KC, P, TC, P).transpose(1, 2, 0, 3)[:, :ntiles]
    return np.ascontiguousarray(xt).astype(dt_np)


def kernel(**inputs):
    query = np.asarray(inputs["query"], dtype=np.float32)
    key = np.asarray(inputs["key"], dtype=np.float32)
    value = np.asarray(inputs["value"], dtype=np.float32)
    vl = np.asarray(inputs["valid_length"]).astype(np.int64)
    W_q = np.asarray(inputs["W_q"], dtype=np.float32)
    b_q = np.asarray(inputs["b_q"], dtype=np.float32)
    W_k = np.asarray(inputs["W_k"], dtype=np.float32)
    b_k = np.asarray(inputs["b_k"], dtype=np.float32)
    W_v = np.asarray(inputs["W_v"], dtype=np.float32)
    b_v = np.asarray(inputs["b_v"], dtype=np.float32)
    W_o = np.asarray(inputs["W_o"], dtype=np.float32)
    b_o = np.asarray(inputs["b_o"], dtype=np.float32)

    B = query.shape[0]
    NB = B
    CPB = (H // NCORES) * DK       # 2 heads per core -> 128 cols
    dt_x = BF16
    dt_np = BF16_NP

    # slot s handles batch order[s]: ascending J with the largest slot
    # second-to-last — small slots first (small startup bubble, and their
    # normalize/out-proj work becomes available early), the largest slot's
    # long exp window then absorbs it all
    Jv = np.where(vl == 0, TC * P, np.minimum(vl, TC * P))
    order = list(np.argsort(Jv, kind="stable"))
    order[-2], order[-1] = order[-1], order[-2]
    J_list = []
    for s in range(NB):
        v = int(vl[order[s]])
        J_list.append(TC if v == 0 else max(1, -(-v // P)))

    nc = _get_nc(NB, J_list, dt_x)

    # host-side shard prep
    xq_np, xk_np, xv_np, mb_np = [], [], [], []
    for s in range(NB):
        b = int(order[s])
        v = int(vl[b])
        J = J_list[s]
        q_b = query[b] if v != 0 else np.zeros_like(query[b])
        xq_np.append(_xt(q_b, dt_np))
        xk_np.append(_xt(key[b], dt_np, J))
        xv_np.append(_xt(value[b], dt_np, J))
        rows = np.arange(P) + (J - 1) * P
        if v == 0:
            m = np.zeros((1, P), np.float32)
        else:
            m = np.where(rows < v, 0.0, 8.0 * MASK_NEG)[None, :]
        mb_np.append(np.ascontiguousarray(m).astype(BF16_NP))

    in_maps = []
    for c in range(NCORES):
        c0 = c * CPB
        cols = slice(c0, c0 + CPB)
        im = {
            "wq": np.ascontiguousarray(
                W_q.reshape(KC, P, H * DK).transpose(1, 0, 2)[:, :, cols]
            ).astype(dt_np),
            "wk": np.ascontiguousarray(
                W_k.reshape(KC, P, H * DK).transpose(1, 0, 2)[:, :, cols]
            ).astype(dt_np),
            "wv": np.ascontiguousarray(
                W_v.reshape(KC, P, H * DK).transpose(1, 0, 2)[:, :, cols]
            ).astype(dt_np),
            "wo": np.ascontiguousarray(
                W_o[cols].reshape(P, KC, P)).astype(dt_np),
            "bq": np.ascontiguousarray(b_q[cols][:, None]).astype(np.float32),
            "bk": np.ascontiguousarray(b_k[cols][:, None]).astype(np.float32),
            "bv": np.ascontiguousarray(b_v[cols][None, :]).astype(dt_np),
        }
        im["onesb"] = np.ones((1, DK), BF16_NP)
        for s in range(NB):
            im[f"xq{s}"] = xq_np[s]
            im[f"xk{s}"] = xk_np[s]
            im[f"xv{s}"] = xv_np[s]
            im[f"mb{s}"] = mb_np[s]
        in_maps.append(im)

    res = run_bass_kernel_spmd(nc, in_maps, list(range(NCORES)))

    out = np.zeros((B, T, D), np.float32)
    for s in range(NB):
        b = int(order[s])
        acc = np.zeros((D, T), np.float32)
        for c in range(NCORES):
            acc += np.asarray(res.results[c][f"o{s}"]).astype(np.float32)
        out[b] = acc.T + b_o[None, :]
    return out


# revision 19
# speedup vs baseline: 1.0467x; 1.0467x over previous
"""Trainium2 Bass kernel for nn_MultiHeadAttention (B=4, T=2048, D=1024,
H=16, d_k=64) on 8 NeuronCores.

Sharding: tensor-parallel over heads — core c computes heads {2c, 2c+1} for
ALL batches (W_q/W_k/W_v column-sharded, W_o row-sharded). The final
all-reduce of the output projection is replaced by a host-side sum of the 8
partial outputs (each written transposed, [D, T]).

Design (evolved across nine profiled hardware iterations):
  - K/V are projected (and their x DMA'd) only for the ceil(vl/128) Tk
    tiles that attention actually reads — 24 of 64 tiles for this input
    distribution (saves ~47us PE + ~21MB HBM per core vs projecting all).
  - scores^T layout (Tk on partitions, Tq free): the two heads' QK^T
    matmuls are K=64 row-tile pairs sharing the PE array; the padding mask
    is premixed into the scores PSUM by a K=1 matmul (mask-row x 8 as lhsT,
    ones as rhs), so every exp() is bias-free and two Tk tiles merge into
    one [128, 2x512] ACT instruction (the 352-cycle ACT overhead was 40% of
    exp cost at single-tile granularity; exp is the critical engine).
  - softmax denominator rides as a ones-column folded into the P@V matmul
    (lhsT = [V_h | 1]); the un-normalized outputs + den rows are staged with
    one [65, 512] DVE copy; reciprocals are batched (bf16 is plenty);
    1/den is broadcast across partitions with a K=1 matmul and applied by
    DVE multiplies.
  - output projection keeps W_o stationary (LDWEIGHTS amortized over 4
    N=512 streams) and writes the partial transposed, [D, T].
  - emission is software-pipelined at instruction granularity: per-engine
    queues are in-order, so projection / normalization / out-projection
    work is pumped in small units between attention iterations, with
    out-projection PSUM->SBUF copies steered to ACT only in exp-sparse
    windows (they otherwise head-block the exp server). Slot order puts
    the second-largest batch first (small startup bubble) and the largest
    second, whose long exp window absorbs the remaining background work.
  - trn2 encodes at most one semaphore wait per instruction; a post-pass
    splits any multi-wait instruction Tile emits into single-wait
    InstEventSemaphore ops (walrus rejects them otherwise).
"""
import os
import sys
from collections import deque

for _p in ("/opt/trn_rl_repo", "/root/.axon_site/_ro/trn_rl_repo"):
    if os.path.isdir(_p) and _p not in sys.path:
        sys.path.append(_p)

import numpy as np
import ml_dtypes

import concourse.bass as bass
import concourse.mybir as mybir
import concourse.tile as tile
from concourse.bass import ts
from concourse.bass_utils import run_bass_kernel_spmd

D = 1024
T = 2048
H = 16
DK = 64
P = 128
KC = D // P          # 8 contraction chunks for the projections
TC = T // P          # 16 token tiles of 128
NT = T // 512        # 4 Tq chunks of 512
NCORES = 8
MASK_NEG = -30000.0

F32 = mybir.dt.float32
F32R = mybir.dt.float32r
BF16 = mybir.dt.bfloat16
AF = mybir.ActivationFunctionType
BF16_NP = ml_dtypes.bfloat16


def _split_multi_waits(nc):
    """trn2 instructions encode at most one sync wait; split the rest into
    standalone single-wait event-semaphore ops."""
    n_split = 0
    for f in nc.m.functions:
        for blk in f.blocks:
            insts = blk.instructions
            out = []
            changed = False
            for inst in insts:
                si = inst.sync_info
                if si is not None and len(si.on_wait) > 1:
                    waits = list(si.on_wait)
                    for k, wt in enumerate(waits[:-1]):
                        ev = mybir.InstEventSemaphore(
                            name=f"{inst.name}_wsplit{k}",
                            engine=inst.engine,
                            ins=[],
                            outs=[],
                            bass_nofuse=True,
                            sync_info=mybir.SyncInfo(on_wait=[wt], on_update=[]),
                        )
                        out.append(ev)
                        n_split += 1
                    inst.sync_info = mybir.SyncInfo(
                        on_wait=[waits[-1]], on_update=si.on_update
                    )
                    changed = True
                out.append(inst)
            if changed:
                blk.instructions = out
    return n_split


def build_nc(NB, J_list, dt_x):
    """Build the SPMD program.

    NB     : number of batch slots handled per core
    J_list : per batch slot, number of 128-row Tk tiles of attention
    dt_x   : dtype of x/weights/intermediates
    """
    nc = bass.Bass()

    # partition-major tile layout: per partition, each 128-token tile is a
    # contiguous [KC, 128] run (2 KB bf16)
    xq_d = [nc.declare_dram_parameter(f"xq{s}", [P, TC, KC, P], dt_x,
                                      isOutput=False) for s in range(NB)]
    xk_d = [nc.declare_dram_parameter(f"xk{s}", [P, J_list[s], KC, P], dt_x,
                                      isOutput=False) for s in range(NB)]
    xv_d = [nc.declare_dram_parameter(f"xv{s}", [P, J_list[s], KC, P], dt_x,
                                      isOutput=False) for s in range(NB)]
    wq_d = nc.declare_dram_parameter("wq", [P, KC, P], dt_x, isOutput=False)
    wk_d = nc.declare_dram_parameter("wk", [P, KC, P], dt_x, isOutput=False)
    wv_d = nc.declare_dram_parameter("wv", [P, KC, P], dt_x, isOutput=False)
    wo_d = nc.declare_dram_parameter("wo", [P, KC, P], dt_x, isOutput=False)
    bq_d = nc.declare_dram_parameter("bq", [P, 1], F32, isOutput=False)
    bk_d = nc.declare_dram_parameter("bk", [P, 1], F32, isOutput=False)
    bv_d = nc.declare_dram_parameter("bv", [1, P], dt_x, isOutput=False)
    mb_d = [nc.declare_dram_parameter(f"mb{s}", [1, P], BF16, isOutput=False)
            for s in range(NB)]
    onesb_d = nc.declare_dram_parameter("onesb", [1, DK], BF16,
                                        isOutput=False)
    o_d = [nc.declare_dram_parameter(f"o{s}", [D, T], BF16, isOutput=True)
           for s in range(NB)]

    with tile.TileContext(nc) as tc:
        with (
            tc.tile_pool(name="pers", bufs=1) as pers,
            tc.tile_pool(name="stream", bufs=3) as stream,
            tc.tile_pool(name="attn", bufs=3) as attn_pool,
            tc.tile_pool(name="small", bufs=3) as small,
            tc.tile_pool(name="outp", bufs=4) as outp,
            tc.tile_pool(name="ps_qk", bufs=2, space="PSUM") as ps_qk,
            tc.tile_pool(name="ps_pv", bufs=2, space="PSUM") as ps_pv,
            tc.tile_pool(name="ps_bg", bufs=2, space="PSUM") as ps_bg,
        ):
            # ---- persistent tensors -------------------------------------
            wq = pers.tile([P, KC, P], dt_x, name="wq")
            wk = pers.tile([P, KC, P], dt_x, name="wk")
            wv = pers.tile([P, KC, P], dt_x, name="wv")
            wo = pers.tile([P, KC, P], dt_x, name="wo")
            bq = pers.tile([P, 1], F32, name="bq")
            bk = pers.tile([P, 1], F32, name="bk")
            bv = pers.tile([1, P], dt_x, name="bv")
            nc.sync.dma_start(wq[:], wq_d[:])
            nc.sync.dma_start(wk[:], wk_d[:])
            nc.sync.dma_start(wv[:], wv_d[:])
            nc.sync.dma_start(wo[:], wo_d[:])
            nc.sync.dma_start(bq[:], bq_d[:])
            nc.sync.dma_start(bk[:], bk_d[:])
            nc.sync.dma_start(bv[:], bv_d[:])
            mb = []
            for s in range(NB):
                t = pers.tile([1, P], BF16, name=f"mb{s}")
                nc.sync.dma_start(t[:], mb_d[s][:])
                mb.append(t)
            ones_r = pers.tile([1, 512], BF16, name="ones_r")
            nc.vector.memset(ones_r[:], 1.0)

            ones_t = pers.tile([1, P], dt_x, name="ones_t")   # V-bias fold lhsT
            nc.vector.memset(ones_t[:], 1.0)
            ones_b = pers.tile([1, DK], BF16, name="ones_b")  # 1/den bcast lhsT
            nc.sync.dma_start(ones_b[:], onesb_d[:])

            QT = [pers.tile([P, T], dt_x, name=f"QT{s}") for s in range(NB)]
            KT = [pers.tile([P, J_list[s] * P], dt_x, name=f"KT{s}")
                  for s in range(NB)]
            # V with a ones column folded in at free index 64 of each head
            V = [pers.tile([P, J_list[s], 2, DK + 1], dt_x, name=f"V{s}")
                 for s in range(NB)]
            for s in range(NB):
                nc.vector.memset(V[s][:, :, :, DK], 1.0)

            AO = [pers.tile([P, T], dt_x, name=f"AO{s}") for s in range(NB)]
            NR = 2 * NT  # unnormalized-output rows per slot (tq, head)
            uo = [pers.tile([DK + 1, NR, 512], BF16, name=f"uo{s}")
                  for s in range(NB)]
            dens = [pers.tile([NR, 512], BF16, name=f"dens{s}")
                    for s in range(NB)]
            recs = [pers.tile([NR, 512], BF16, name=f"rec{s}")
                    for s in range(NB)]

            # ---- emission-unit generators -------------------------------
            def proj_gen(s):
                J = J_list[s]
                # K projection: weight-stationary per kc over <=4-tile chunks
                ngr = -(-J // 4)
                for g in range(ngr):
                    t0 = 4 * g
                    nt_ = min(4, J - t0)
                    xkw = stream.tile([P, 4, KC, P], dt_x, tag="xk_w")
                    nc.sync.dma_start(xkw[:, 0:nt_], xk_d[s][:, t0:t0 + nt_])
                    ps_k = ps_bg.tile([P, 512], F32, tag="bg")
                    for kc in range(KC):
                        nc.tensor.matmul(ps_k[:, 0:nt_ * P], wk[:, kc, :],
                                         xkw[:, 0:nt_, kc, :],
                                         start=(kc == 0), stop=(kc == KC - 1))
                        if kc == 3:
                            yield
                    nc.vector.tensor_scalar_add(
                        KT[s][:, t0 * P:(t0 + nt_) * P],
                        ps_k[:, 0:nt_ * P], bk[:, 0:1])
                    yield
                # V projection: x-tile stationary, wv moving (N=128)
                for g in range(ngr):
                    t0 = 4 * g
                    nt_ = min(4, J - t0)
                    xvw = stream.tile([P, 4, KC, P], dt_x, tag="xv_w")
                    nc.sync.dma_start(xvw[:, 0:nt_], xv_d[s][:, t0:t0 + nt_])
                    ps_v = ps_bg.tile([P, 512], F32, tag="bg")
                    for i in range(nt_):
                        reg = ps_v[:, ts(i, P)]
                        for kc in range(KC):
                            nc.tensor.matmul(reg, xvw[:, i, kc, :],
                                             wv[:, kc, :],
                                             start=(kc == 0), stop=False)
                        nc.tensor.matmul(reg, ones_t[0:1, :], bv[0:1, :],
                                         start=False, stop=True)
                        yield
                    for i in range(nt_):
                        nc.vector.tensor_copy(
                            V[s][:, t0 + i, :, 0:DK],
                            ps_v[:, ts(i, P)].rearrange("p (h d) -> p h d",
                                                        d=DK))
                    yield
                # Q projection: weight-stationary per kc over 4-tile chunks
                for g in range(NT):
                    xqw = stream.tile([P, 4, KC, P], dt_x, tag="xq_w")
                    nc.sync.dma_start(xqw[:], xq_d[s][:, 4 * g:4 * g + 4])
                    ps_q = ps_bg.tile([P, 512], F32, tag="bg")
                    for kc in range(KC):
                        nc.tensor.matmul(ps_q[:], wq[:, kc, :],
                                         xqw[:, :, kc, :],
                                         start=(kc == 0), stop=(kc == KC - 1))
                        if kc == 3:
                            yield
                    nc.vector.tensor_scalar_add(QT[s][:, ts(g, 512)],
                                                ps_q[:], bq[:, 0:1])
                    yield

            def attn_emit(s, pump):
                J = J_list[s]
                # j-tile pair schedule (the last tile's mask is premixed
                # into its psum, so it merges like any interior tile)
                items = [tuple(range(j, min(j + 2, J)))
                         for j in range(0, J, 2)]
                for tq in range(NT):
                    ps_os = [ps_pv.tile([P, 512], F32, tag="pv",
                                        name=f"pv{h}")
                             for h in range(2)]
                    # unit = (item, head): emit QK+exp for unit u, then the
                    # PV of unit u-1, so the PE never heads-of-line-waits on
                    # an exp (per-engine queues are in-order)
                    pv_pending = deque()
                    for it in items:
                        for h in range(2):
                            pss = ps_qk.tile([P, 2, 512], F32, tag="qk")
                            for k, j_ in enumerate(it):
                                masked = j_ == J - 1
                                nc.tensor.matmul(
                                    pss[:, k, :],
                                    KT[s][ts(h, DK), ts(j_, P)],
                                    QT[s][ts(h, DK), ts(tq, 512)],
                                    start=True, stop=not masked,
                                    tile_position=(h * DK, 0))
                                if masked:
                                    nc.tensor.matmul(
                                        pss[:, k, :], mb[s][0:1, :],
                                        ones_r[0:1, :],
                                        start=False, stop=True)
                            at = attn_pool.tile([P, 2, 512], dt_x, tag="at")
                            if len(it) == 2:
                                nc.scalar.activation(at[:, :, :], pss[:],
                                                     AF.Exp, scale=0.125)
                            else:
                                nc.scalar.activation(at[:, 0, :],
                                                     pss[:, 0, :], AF.Exp,
                                                     scale=0.125)
                            for k, j_ in enumerate(it):
                                pv_pending.append((j_, h, at, k))
                            while len(pv_pending) > 2:
                                _emit_pv(s, tq, ps_os, pv_pending.popleft())
                            pump()
                    while pv_pending:
                        _emit_pv(s, tq, ps_os, pv_pending.popleft())
                    for h in range(2):
                        r = tq * 2 + h
                        # den row rides along at partition DK (bf16 is
                        # plenty for the softmax denominator)
                        nc.vector.tensor_copy(uo[s][:, r, :],
                                              ps_os[h][0:DK + 1, :])
                        nc.sync.dma_start(dens[s][r:r + 1, :],
                                          uo[s][DK:DK + 1, r, :])
                    pump()

            def _emit_pv(s, tq, ps_os, unit):
                J = J_list[s]
                j_, h, at, k = unit
                nc.tensor.matmul(ps_os[h][0:DK + 1, :],
                                 V[s][:, j_, h, :], at[:, k, :],
                                 start=(j_ == 0), stop=(j_ == J - 1))

            def norm_gen(s):
                # batched normalization (bf16 denominators are plenty)
                with nc.allow_low_precision(reason="bf16 1/den is plenty"):
                    nc.vector.reciprocal(recs[s][:], dens[s][:])
                yield
                for r in range(NR):
                    # stage rec row at partition 0 for the K=1 bcast matmul
                    rst = small.tile([1, 512], BF16, tag="rst")
                    nc.sync.dma_start(rst[:], recs[s][r:r + 1, :])
                    ps_b = ps_bg.tile([P, 512], F32, tag="bg")
                    nc.tensor.matmul(ps_b[0:DK, :], ones_b[0:1, :],
                                     rst[0:1, :], start=True, stop=True)
                    h = r % 2
                    tq = r // 2
                    nc.vector.tensor_mul(
                        out=AO[s][ts(h, DK), ts(tq, 512)],
                        in0=ps_b[0:DK, :], in1=uo[s][0:DK, r, :])
                    yield

            def outproj_gen(s, use_act):
                # output projection, pumped into later attention windows;
                # copies go to ACT only where the exp stream is sparse
                unit = 0
                for dt_i in range(KC):
                    for n in range(NT):
                        ps_op = ps_bg.tile([P, 512], F32, tag="bg")
                        nc.tensor.matmul(ps_op[:], wo[:, dt_i, :],
                                         AO[s][:, ts(n, 512)],
                                         start=True, stop=True)
                        ot = outp.tile([P, 512], BF16, tag="ot")
                        if use_act and unit % 2 == 0:
                            nc.scalar.activation(ot[:], ps_op[:],
                                                 AF.Identity)
                        else:
                            nc.vector.tensor_copy(ot[:], ps_op[:])
                        nc.sync.dma_start(o_d[s][ts(dt_i, P), ts(n, 512)],
                                          ot[:])
                        unit += 1
                        yield

            # ---- software-pipelined emission ----------------------------
            pending = deque()

            def pump(n=2):
                k = 0
                while pending and k < n:
                    try:
                        next(pending[0][1])
                        k += 1
                    except StopIteration:
                        pending.popleft()

            def drain_proj():
                while pending and pending[0][0] == "proj":
                    try:
                        next(pending[0][1])
                    except StopIteration:
                        pending.popleft()

            for u in proj_gen(0):
                pass
            for s in range(NB):
                if s + 1 < NB:
                    pending.append(("proj", proj_gen(s + 1)))
                if s >= 1:
                    pending.append(("norm", norm_gen(s - 1)))
                    # ACT helps with copies only in exp-sparse windows
                    pending.append(("op", outproj_gen(s - 1,
                                                     J_list[s] < 8)))
                attn_emit(s, pump)
                drain_proj()
            pending.append(("norm", norm_gen(NB - 1)))
            pending.append(("op", outproj_gen(NB - 1, True)))
            while pending:
                pump(1000)

    _split_multi_waits(nc)
    return nc


_CACHE = {}


def _get_nc(NB, J_list, dt_x):
    key = (NB, tuple(J_list), str(dt_x))
    if key not in _CACHE:
        _CACHE[key] = build_nc(NB, J_list, dt_x)
    return _CACHE[key]


def _xt(x, dt_np, ntiles=TC):
    """[T, D] -> [P, ntiles, KC, 128] partition-major tile layout."""
    xt = x.T.reshape(KC, P, TC, P).transpose(1, 2, 0, 3)[:, :ntiles]
    return np.ascontiguousarray(xt).astype(dt_np)


def kernel(**inputs):
    query = np.asarray(inputs["query"], dtype=np.float32)
    key = np.asarray(inputs["key"], dtype=np.float32)
    value = np.asarray(inputs["value"], dtype=np.float32)
    vl = np.asarray(inputs["valid_length"]).astype(np.int64)
    W_q = np.asarray(inputs["W_q"], dtype=np.float32)
    b_q = np.asarray(inputs["b_q"], dtype=np.float32)
    W_k = np.asarray(inputs["W_k"], dtype=np.float32)
    b_k = np.asarray(inputs["b_k"], dtype=np.float32)
    W_v = np.asarray(inputs["W_v"], dtype=np.float32)
    b_v = np.asarray(inputs["b_v"], dtype=np.float32)
    W_o = np.asarray(inputs["W_o"], dtype=np.float32)
    b_o = np.asarray(inputs["b_o"], dtype=np.float32)

    B = query.shape[0]
    NB = B
    CPB = (H // NCORES) * DK       # 2 heads per core -> 128 cols
    dt_x = BF16
    dt_np = BF16_NP

    # slot s handles batch order[s]: second-largest first (small-ish
    # startup bubble), then the largest (its long exp window absorbs the
    # later projections and copies), then descending
    Jv = np.where(vl == 0, TC * P, np.minimum(vl, TC * P))
    order = list(np.argsort(-Jv, kind="stable"))
    order[0], order[1] = order[1], order[0]
    J_list = []
    for s in range(NB):
        v = int(vl[order[s]])
        J_list.append(TC if v == 0 else max(1, -(-v // P)))

    nc = _get_nc(NB, J_list, dt_x)

    # host-side shard prep
    xq_np, xk_np, xv_np, mb_np = [], [], [], []
    for s in range(NB):
        b = int(order[s])
        v = int(vl[b])
        J = J_list[s]
        q_b = query[b] if v != 0 else np.zeros_like(query[b])
        xq_np.append(_xt(q_b, dt_np))
        xk_np.append(_xt(key[b], dt_np, J))
        xv_np.append(_xt(value[b], dt_np, J))
        rows = np.arange(P) + (J - 1) * P
        if v == 0:
            m = np.zeros((1, P), np.float32)
        else:
            m = np.where(rows < v, 0.0, 8.0 * MASK_NEG)[None, :]
        mb_np.append(np.ascontiguousarray(m).astype(BF16_NP))

    in_maps = []
    for c in range(NCORES):
        c0 = c * CPB
        cols = slice(c0, c0 + CPB)
        im = {
            "wq": np.ascontiguousarray(
                W_q.reshape(KC, P, H * DK).transpose(1, 0, 2)[:, :, cols]
            ).astype(dt_np),
            "wk": np.ascontiguousarray(
                W_k.reshape(KC, P, H * DK).transpose(1, 0, 2)[:, :, cols]
            ).astype(dt_np),
            "wv": np.ascontiguousarray(
                W_v.reshape(KC, P, H * DK).transpose(1, 0, 2)[:, :, cols]
            ).astype(dt_np),
            "wo": np.ascontiguousarray(
                W_o[cols].reshape(P, KC, P)).astype(dt_np),
            "bq": np.ascontiguousarray(b_q[cols][:, None]).astype(np.float32),
            "bk": np.ascontiguousarray(b_k[cols][:, None]).astype(np.float32),
            "bv": np.ascontiguousarray(b_v[cols][None, :]).astype(dt_np),
        }
        im["onesb"] = np.ones((1, DK), BF16_NP)
        for s in range(NB):
            im[f"xq{s}"] = xq_np[s]
            im[f"xk{s}"] = xk_np[s]
            im[f"xv{s}"] = xv_np[s]
            im[f"mb{s}"] = mb_np[s]
        in_maps.append(im)

    res = run_bass_kernel_spmd(nc, in_maps, list(range(NCORES)))

    out = np.zeros((B, T, D), np.float32)
    for s in range(NB):
        b = int(order[s])
        acc = np.zeros((D, T), np.float32)
        for c in range(NCORES):
            acc += np.asarray(res.results[c][f"o{s}"]).astype(np.float32)
        out[b] = acc.T + b_o[None, :]
    return out
